# revision 1
# baseline (speedup 1.0000x reference)
"""Bass/Trainium2 kernel for a fused GRU cell.

  r   = sigmoid(x @ W_ir.T + h @ W_hr.T + b_r)
  z   = sigmoid(x @ W_iz.T + h @ W_hz.T + b_z)
  g   = tanh  (x @ W_ih.T + (r*h) @ W_hh.T + b_h)
  h_t = (1-z)*h + z*g

Sharding: data-parallel over the batch (8192 -> 1024 rows per core on 8
NeuronCores), weights replicated, no collectives.

Everything on-device is computed in a transposed layout ([hidden, batch]
with hidden on SBUF partitions) so that
  - the per-h-tile bias is a per-partition scalar (free with activation),
  - weight tiles land as natural [K,M] stationary operands,
  - all DMAs are contiguous (host numpy does every transpose/reshape).
Matmuls run as float32r (full PE rate, ~13-bit mantissa) accumulating in
fp32 PSUM; activations run in-place on PSUM.
"""

import sys

for _p in ("/opt/trn_rl_repo", "/root/.axon_site/_ro/trn_rl_repo"):
    if _p not in sys.path:
        sys.path.append(_p)

import numpy as np

P = 128          # SBUF partitions
BC_MAX = 512     # fp32 moving-operand / PSUM-bank max free dim
N_CORES = 8

_PROG_CACHE = {}


def _pick_qt(nj):
    for qt in (4, 6, 3, 2, 1):
        if nj % qt == 0:
            return qt
    return 1


def build_program(Bc, IN, H):
    """Build the per-core SPMD Bass program (identical on all cores)."""
    from contextlib import ExitStack

    from concourse import bacc, bass, mybir, tile
    from concourse.dt import dt

    KI, KH, NT = IN // P, H // P, H // P
    NJ = KI + KH                 # contraction tiles per gate per h-tile
    QT = _pick_qt(NJ)            # weight tiles per DMA slab
    NQ = NJ // QT
    BC = min(BC_MAX, Bc)
    NB = Bc // BC
    f32, f32r = dt.float32, dt.float32r
    SIG = mybir.ActivationFunctionType.Sigmoid
    TANH = mybir.ActivationFunctionType.Tanh

    nc = bacc.Bacc("TRN2", debug=False)
    xt_d = nc.declare_dram_parameter("xt", [P, KI, Bc], f32r, False)
    hp_d = nc.declare_dram_parameter("hp", [P, KH, Bc], f32r, False)
    wr_d = nc.declare_dram_parameter("wr", [NT, NQ, P, QT * P], f32r, False)
    wz_d = nc.declare_dram_parameter("wz", [NT, NQ, P, QT * P], f32r, False)
    wh_d = nc.declare_dram_parameter("wh", [NT, NQ, P, QT * P], f32r, False)
    b_d = nc.declare_dram_parameter("bias", [P, NT * 3], f32, False)
    out_d = nc.declare_dram_parameter("out", [NT, P, Bc], f32, True)

    with ExitStack() as ctx:
        tc = ctx.enter_context(tile.TileContext(nc))
        res = ctx.enter_context(tc.tile_pool(name="res", bufs=1))
        wp = ctx.enter_context(tc.tile_pool(name="wp", bufs=8))
        pp = ctx.enter_context(
            tc.tile_pool(name="pp", bufs=4, space=bass.MemorySpace.PSUM)
        )
        op = ctx.enter_context(tc.tile_pool(name="op", bufs=2))
        zp = ctx.enter_context(tc.tile_pool(name="zp", bufs=2))

        xt = res.tile([P, KI, Bc], f32r, tag="xt")
        hp = res.tile([P, KH, Bc], f32r, tag="hp")
        rh = res.tile([P, KH, Bc], f32r, tag="rh")
        bias = res.tile([P, NT * 3], f32, tag="bias")

        nc.sync.dma_start(out=bias[:], in_=b_d[:])
        for j in range(KI):
            nc.sync.dma_start(out=xt[:, j, :], in_=xt_d[:, j, :])
        for t in range(KH):
            nc.sync.dma_start(out=hp[:, t, :], in_=hp_d[:, t, :])

        def gate(ps, w_d, hti, srch):
            # ps[:, bc] += sum_j W_tile[j].T @ moving[j][:, bc]
            for q in range(NQ):
                slab = wp.tile([P, QT * P], f32r, tag="w")
                nc.sync.dma_start(out=slab[:], in_=w_d[hti, q])
                for jj in range(QT):
                    j = q * QT + jj
                    lhs = slab[:, jj * P : (jj + 1) * P]
                    mov = xt[:, j, :] if j < KI else srch[:, j - KI, :]
                    for bc in range(NB):
                        nc.tensor.matmul(
                            ps[:, bc * BC : (bc + 1) * BC],
                            lhs,
                            mov[:, bc * BC : (bc + 1) * BC],
                            start=(j == 0),
                            stop=(j == NJ - 1),
                            skip_group_check=True,
                        )

        # ---- phase R: r = sigmoid(gi_r + gh_r + b_r); rh = r * h ----
        for hti in range(NT):
            ps = pp.tile([P, Bc], f32, tag="ps")
            gate(ps, wr_d, hti, hp)
            for bc in range(NB):
                sl = slice(bc * BC, (bc + 1) * BC)
                nc.scalar.activation(
                    ps[:, sl], ps[:, sl], SIG, bias=bias[:, hti * 3 : hti * 3 + 1]
                )
                nc.vector.tensor_mul(rh[:, hti, sl], ps[:, sl], hp[:, hti, sl])

        # ---- phase ZH: z, g, h_t = h + z*(g - h) ----
        for hti in range(NT):
            psz = pp.tile([P, Bc], f32, tag="ps")
            gate(psz, wz_d, hti, hp)
            psh = pp.tile([P, Bc], f32, tag="ps")
            gate(psh, wh_d, hti, rh)
            for bc in range(NB):
                sl = slice(bc * BC, (bc + 1) * BC)
                nc.scalar.activation(
                    psz[:, sl], psz[:, sl], SIG, bias=bias[:, hti * 3 + 1 : hti * 3 + 2]
                )
                nc.scalar.activation(
                    psh[:, sl], psh[:, sl], TANH, bias=bias[:, hti * 3 + 2 : hti * 3 + 3]
                )
                # DVE may read only ONE psum operand per instruction:
                # stage z into SBUF first
                zs = zp.tile([P, BC], f32, tag="zs")
                nc.vector.tensor_scalar_add(zs[:], psz[:, sl], 0.0)
                nc.vector.tensor_sub(psh[:, sl], psh[:, sl], hp[:, hti, sl])
                nc.vector.tensor_mul(psh[:, sl], zs[:], psh[:, sl])
                o = op.tile([P, BC], f32, tag="o")
                nc.vector.tensor_add(o[:], psh[:, sl], hp[:, hti, sl])
                nc.gpsimd.dma_start(out=out_d[hti, :, sl], in_=o[:])

    nc.compile()
    return nc


def _pack_weight_gate(Wi, Wh, QT):
    """Stack [Wi-tiles; Wh-tiles] -> (NT, NQ, P, QT*P) DMA-slab layout.

    slab[hti, q][p, jj*P + m] = W[hti*P + m, k] with k = (q*QT+jj tile)*P + p,
    i.e. each 128x128 stationary tile is W.T for that (k-tile, h-tile) block.
    """
    H, IN = Wi.shape
    KI, KH, NT = IN // P, H // P, H // P
    ti = Wi.reshape(NT, P, KI, P).transpose(0, 2, 3, 1)  # (NT, KI, p, m)
    th = Wh.reshape(NT, P, KH, P).transpose(0, 2, 3, 1)  # (NT, KH, p, m)
    cat = np.concatenate([ti, th], axis=1)               # (NT, NJ, p, m)
    NJ = KI + KH
    NQ = NJ // QT
    return np.ascontiguousarray(
        cat.reshape(NT, NQ, QT, P, P).transpose(0, 1, 3, 2, 4).reshape(NT, NQ, P, QT * P)
    )


def _pack_acts(a):
    """(Bc, D) -> (P, D//P, Bc) with [p, t, b] = a[b, t*P + p]."""
    Bc, D = a.shape
    return np.ascontiguousarray(a.T.reshape(D // P, P, Bc).transpose(1, 0, 2))


def run(x_t, h_prev, W_ir, W_iz, W_ih, W_hr, W_hz, W_hh, b_r, b_z, b_h,
        trace=False):
    from concourse.bass_utils import run_bass_kernel_spmd

    x_t = np.asarray(x_t, dtype=np.float32)
    h_prev = np.asarray(h_prev, dtype=np.float32)
    B, IN = x_t.shape
    H = h_prev.shape[1]
    assert B % N_CORES == 0
    Bc = B // N_CORES
    NT = H // P
    QT = _pick_qt(IN // P + H // P)

    key = (Bc, IN, H)
    if key not in _PROG_CACHE:
        _PROG_CACHE[key] = build_program(Bc, IN, H)
    nc = _PROG_CACHE[key]

    wr = _pack_weight_gate(np.asarray(W_ir, np.float32), np.asarray(W_hr, np.float32), QT)
    wz = _pack_weight_gate(np.asarray(W_iz, np.float32), np.asarray(W_hz, np.float32), QT)
    wh = _pack_weight_gate(np.asarray(W_ih, np.float32), np.asarray(W_hh, np.float32), QT)
    bias = np.ascontiguousarray(
        np.stack(
            [np.asarray(b_r, np.float32), np.asarray(b_z, np.float32),
             np.asarray(b_h, np.float32)], axis=-1
        ).reshape(NT, P, 3).transpose(1, 0, 2).reshape(P, NT * 3)
    )

    in_maps = []
    for c in range(N_CORES):
        rows = slice(c * Bc, (c + 1) * Bc)
        in_maps.append({
            "xt": _pack_acts(x_t[rows]),
            "hp": _pack_acts(h_prev[rows]),
            "wr": wr, "wz": wz, "wh": wh, "bias": bias,
        })

    kw = {}
    if trace:
        kw = dict(trace=True, trace_cores=[0])
    res = run_bass_kernel_spmd(nc, in_maps, core_ids=list(range(N_CORES)), **kw)

    outs = []
    for c in range(N_CORES):
        o = res.results[c]["out"]          # (NT, P, Bc)
        outs.append(o.reshape(H, Bc).T)    # (Bc, H)
    full = np.concatenate(outs, axis=0).astype(np.float32)
    return (full, res) if trace else full


def kernel(**inputs):
    return run(**inputs)



# revision 3
# speedup vs baseline: 1.0882x; 1.0882x over previous
"""Bass/Trainium2 kernel for a fused GRU cell.

  r   = sigmoid(x @ W_ir.T + h @ W_hr.T + b_r)
  z   = sigmoid(x @ W_iz.T + h @ W_hz.T + b_z)
  g   = tanh  (x @ W_ih.T + (r*h) @ W_hh.T + b_h)
  h_t = (1-z)*h + z*g

Sharding: data-parallel over the batch (8192 -> 1024 rows per core on 8
NeuronCores), weights replicated, no collectives.

Layout: transposed on-device ([hidden, batch] with hidden on SBUF
partitions) so biases are per-partition scalars and weight tiles land as
natural [K,M] stationary operands. All matmul operands are bf16 (same
1 cycle/row PE rate as float32r but half the DMA/LDWEIGHTS bytes);
accumulation is fp32 PSUM, activations and the final combine are fp32.

Input DMAs are spread across engine queues (x on scalar, h on vector,
weights on sync, outputs on gpsimd) and the first few h-tiles run their
x-side partial sums first, so the PE starts ~2us in instead of waiting
for the full x+h load.
"""

import sys

for _p in ("/opt/trn_rl_repo", "/root/.axon_site/_ro/trn_rl_repo"):
    if _p not in sys.path:
        sys.path.append(_p)

import numpy as np
import ml_dtypes

P = 128          # SBUF partitions
BC = 512         # moving free-dim per matmul (one fp32 PSUM bank)
QT = 8           # 128x128 k-tiles per weight DMA slab
N_CORES = 8

_PROG_CACHE = {}


def build_program(Bc, IN, H):
    """Build the per-core SPMD Bass program (identical on all cores)."""
    from contextlib import ExitStack

    from concourse import bacc, bass, mybir, tile
    from concourse.dt import dt

    KI, KH, NT = IN // P, H // P, H // P
    NJ = KI + KH                 # contraction tiles per gate per h-tile
    assert NJ % QT == 0
    NQ = NJ // QT
    NB = Bc // BC
    f32, bf16 = dt.float32, dt.bfloat16
    SIG = mybir.ActivationFunctionType.Sigmoid
    TANH = mybir.ActivationFunctionType.Tanh

    nc = bacc.Bacc("TRN2", debug=False)
    xt_d = nc.declare_dram_parameter("xt", [P, KI, Bc], bf16, False)
    hp_d = nc.declare_dram_parameter("hp", [P, KH, Bc], bf16, False)
    wr_d = nc.declare_dram_parameter("wr", [NT, NQ, P, QT * P], bf16, False)
    wz_d = nc.declare_dram_parameter("wz", [NT, NQ, P, QT * P], bf16, False)
    wh_d = nc.declare_dram_parameter("wh", [NT, NQ, P, QT * P], bf16, False)
    b_d = nc.declare_dram_parameter("bias", [P, NT * 3], f32, False)
    out_d = nc.declare_dram_parameter("out", [NT, P, Bc], f32, True)

    with ExitStack() as ctx:
        tc = ctx.enter_context(tile.TileContext(nc))
        res = ctx.enter_context(tc.tile_pool(name="res", bufs=1))
        wp = ctx.enter_context(tc.tile_pool(name="wp", bufs=18))
        pp = ctx.enter_context(
            tc.tile_pool(name="pp", bufs=8, space=bass.MemorySpace.PSUM)
        )
        op = ctx.enter_context(tc.tile_pool(name="op", bufs=4))
        zp = ctx.enter_context(tc.tile_pool(name="zp", bufs=4))

        xt = res.tile([P, KI, Bc], bf16, tag="xt")
        hp = res.tile([P, KH, Bc], bf16, tag="hp")
        rh = res.tile([P, KH, Bc], bf16, tag="rh")
        bias = res.tile([P, NT * 3], f32, tag="bias")

        # x+bias on the scalar queue, h on the gpsimd queue (its output
        # DMAs only start much later), weights on sync: the three input
        # streams overlap from t=0.
        nc.scalar.dma_start(out=bias[:], in_=b_d[:])
        for j in range(KI):
            nc.scalar.dma_start(out=xt[:, j, :], in_=xt_d[:, j, :])
        for t in range(KH):
            nc.gpsimd.dma_start(out=hp[:, t, :], in_=hp_d[:, t, :])

        def slab_dma(w_d, hti, q):
            s = wp.tile([P, QT * P], bf16, tag="w")
            nc.sync.dma_start(out=s[:], in_=w_d[hti, q])
            return s

        def mm_run(ps, slabs, bc, srch, js, start, stop):
            sl = slice(bc * BC, (bc + 1) * BC)
            last = js[-1]
            first = js[0]
            for j in js:
                q, jj = divmod(j, QT)
                lhs = slabs[q][:, jj * P : (jj + 1) * P]
                mov = xt[:, j, sl] if j < KI else srch[:, j - KI, sl]
                nc.tensor.matmul(
                    ps[:],
                    lhs,
                    mov,
                    start=(start and j == first),
                    stop=(stop and j == last),
                    skip_group_check=True,
                )

        def r_epilogue(ps, hti, bc):
            sl = slice(bc * BC, (bc + 1) * BC)
            nc.scalar.activation(
                ps[:], ps[:], SIG, bias=bias[:, hti * 3 : hti * 3 + 1]
            )
            nc.vector.tensor_mul(rh[:, hti, sl], ps[:], hp[:, hti, sl])

        # ---- phase R: r = sigmoid(gi_r + gh_r + b_r); rh = r * h ----
        # The first PRE h-tiles run their x-side partial sums before any
        # h-side work, so the PE has ~PRE*KI*NB matmuls to chew on while
        # the h_prev tiles are still arriving.
        PRE = min(4, NT, (8 // NB) if NB else NT)
        r_slabs = {hti: [slab_dma(wr_d, hti, 0)] for hti in range(PRE)}
        r_ps = {}
        for hti in range(PRE):
            for bc in range(NB):
                ps = pp.tile([P, BC], f32, tag="ps")
                r_ps[(hti, bc)] = ps
                mm_run(ps, r_slabs[hti], bc, hp, list(range(KI)),
                       start=True, stop=False)
        for hti in range(PRE):
            r_slabs[hti].extend(slab_dma(wr_d, hti, q) for q in range(1, NQ))
        for hti in range(PRE):
            for bc in range(NB):
                ps = r_ps[(hti, bc)]
                mm_run(ps, r_slabs[hti], bc, hp, list(range(KI, NJ)),
                       start=False, stop=True)
                r_epilogue(ps, hti, bc)
        for hti in range(PRE, NT):
            slabs = [slab_dma(wr_d, hti, q) for q in range(NQ)]
            for bc in range(NB):
                ps = pp.tile([P, BC], f32, tag="ps")
                mm_run(ps, slabs, bc, hp, list(range(NJ)), start=True, stop=True)
                r_epilogue(ps, hti, bc)

        # ---- phase ZH: z, g, h_t = h + z*(g - h) ----
        for hti in range(NT):
            zslabs = [slab_dma(wz_d, hti, q) for q in range(NQ)]
            hslabs = [slab_dma(wh_d, hti, q) for q in range(NQ)]
            for bc in range(NB):
                sl = slice(bc * BC, (bc + 1) * BC)
                psz = pp.tile([P, BC], f32, tag="ps")
                mm_run(psz, zslabs, bc, hp, list(range(NJ)), start=True, stop=True)
                psh = pp.tile([P, BC], f32, tag="ps")
                mm_run(psh, hslabs, bc, rh, list(range(NJ)), start=True, stop=True)
                # z to SBUF (frees the psz bank; also keeps every DVE op at
                # a single PSUM operand), tanh in place on PSUM.
                zs = zp.tile([P, BC], f32, tag="zs")
                nc.scalar.activation(
                    zs[:], psz[:], SIG, bias=bias[:, hti * 3 + 1 : hti * 3 + 2]
                )
                nc.scalar.activation(
                    psh[:], psh[:], TANH, bias=bias[:, hti * 3 + 2 : hti * 3 + 3]
                )
                nc.vector.tensor_sub(psh[:], psh[:], hp[:, hti, sl])
                nc.vector.tensor_mul(psh[:], zs[:], psh[:])
                o = op.tile([P, BC], f32, tag="o")
                nc.vector.tensor_add(o[:], psh[:], hp[:, hti, sl])
                nc.gpsimd.dma_start(out=out_d[hti, :, sl], in_=o[:])

    nc.compile()
    return nc


def _pack_weight_gate(Wi, Wh):
    """Stack [Wi-tiles; Wh-tiles] -> (NT, NQ, P, QT*P) DMA-slab layout.

    slab[hti, q][p, jj*P + m] = W[hti*P + m, k] with k = (q*QT+jj tile)*P + p,
    i.e. each 128x128 stationary tile is W.T for that (k-tile, h-tile) block.
    """
    H, IN = Wi.shape
    KI, KH, NT = IN // P, H // P, H // P
    ti = Wi.reshape(NT, P, KI, P).transpose(0, 2, 3, 1)  # (NT, KI, p, m)
    th = Wh.reshape(NT, P, KH, P).transpose(0, 2, 3, 1)  # (NT, KH, p, m)
    cat = np.concatenate([ti, th], axis=1)               # (NT, NJ, p, m)
    NJ = KI + KH
    NQ = NJ // QT
    return np.ascontiguousarray(
        cat.reshape(NT, NQ, QT, P, P).transpose(0, 1, 3, 2, 4)
        .reshape(NT, NQ, P, QT * P)
    ).astype(ml_dtypes.bfloat16)


def _pack_acts(a):
    """(Bc, D) -> (P, D//P, Bc) bf16 with [p, t, b] = a[b, t*P + p]."""
    Bc, D = a.shape
    return np.ascontiguousarray(
        a.T.reshape(D // P, P, Bc).transpose(1, 0, 2)
    ).astype(ml_dtypes.bfloat16)


def run(x_t, h_prev, W_ir, W_iz, W_ih, W_hr, W_hz, W_hh, b_r, b_z, b_h,
        trace=False):
    from concourse.bass_utils import run_bass_kernel_spmd

    x_t = np.asarray(x_t, dtype=np.float32)
    h_prev = np.asarray(h_prev, dtype=np.float32)
    B, IN = x_t.shape
    H = h_prev.shape[1]
    assert B % N_CORES == 0
    Bc = B // N_CORES
    NT = H // P

    key = (Bc, IN, H)
    if key not in _PROG_CACHE:
        _PROG_CACHE[key] = build_program(Bc, IN, H)
    nc = _PROG_CACHE[key]

    wr = _pack_weight_gate(np.asarray(W_ir, np.float32), np.asarray(W_hr, np.float32))
    wz = _pack_weight_gate(np.asarray(W_iz, np.float32), np.asarray(W_hz, np.float32))
    wh = _pack_weight_gate(np.asarray(W_ih, np.float32), np.asarray(W_hh, np.float32))
    bias = np.ascontiguousarray(
        np.stack(
            [np.asarray(b_r, np.float32), np.asarray(b_z, np.float32),
             np.asarray(b_h, np.float32)], axis=-1
        ).reshape(NT, P, 3).transpose(1, 0, 2).reshape(P, NT * 3)
    )

    in_maps = []
    for c in range(N_CORES):
        rows = slice(c * Bc, (c + 1) * Bc)
        in_maps.append({
            "xt": _pack_acts(x_t[rows]),
            "hp": _pack_acts(h_prev[rows]),
            "wr": wr, "wz": wz, "wh": wh, "bias": bias,
        })

    kw = {}
    if trace:
        kw = dict(trace=True, trace_cores=[0])
    res = run_bass_kernel_spmd(nc, in_maps, core_ids=list(range(N_CORES)), **kw)

    outs = []
    for c in range(N_CORES):
        o = res.results[c]["out"]          # (NT, P, Bc)
        outs.append(o.reshape(H, Bc).T)    # (Bc, H)
    full = np.concatenate(outs, axis=0).astype(np.float32)
    return (full, res) if trace else full


def kernel(**inputs):
    return run(**inputs)
